# revision 35
# baseline (speedup 1.0000x reference)
"""AutoRound/GPTQ int4 linear on 8 Trainium2 NeuronCores.

y = x @ dequant(qweight, qzeros, scales), computed in bf16 like the torch
module: deq = (w_int4 - zeros[g]) * scales[g] in fp32, cast to bf16;
y = bf16_matmul(x.bf16, deq.bf16) with fp32 accumulation, output cast
back to fp32.

Sharding: 8-way tensor-parallel on out_features (512 per core), x
replicated. Each core dequantizes its weight slice on-chip and computes
a [512 out, 8192 tok] bf16 transposed output slice; the host
reassembles. (TP8 keeps the dequant stream far ahead of the PE; x DMA
at 134 MB/core still fits under the 437us matmul window.)

Device-side layout tricks:
- The contraction (in_features) index is interleaved so that SBUF
  k-chunk `cc = blk*8 + j` holds k = blk*1024 + 8*p + j at partition p.
  Nibble j of packed qweight row p (of the block's 128 rows) is then
  exactly the weight for partition p of chunk cc, so the int4 unpack is
  one fused shift+mask tensor_scalar per chunk with a *constant* shift.
  The host feeds x^T with rows permuted the same way so the matmul
  contraction stays consistent.
- qweight is split on the host into int16 low/high planes so the whole
  dequant chain runs in 16-bit DVE fast modes: extract at 4x, subtract
  (int16-int16 -> bf16) and scale-multiply (bf16*fp16 -> bf16) at 2x.
- zeros are unpacked and, like scales, group-replicated x16 on the host
  (tiny metadata) so each block needs just three plain 128-partition
  loads instead of many small broadcast DMAs (SP issue is ~0.6us/DMA).
- x is cast fp32 -> bf16 inline by SWDGE (gpsimd) converting DMAs, which
  round-to-nearest-even exactly like the reference's astype(bf16).
- A short dummy-matmul warmup keeps the PE HAM clock-gate at 2.4 GHz
  through the dequant window, and each output group's k-accumulation
  order is rotated so PSUM groups chase the dequant frontier instead of
  all stalling on the last-produced chunk.
"""

import numpy as np
import ml_dtypes

PACK = 8
IN_F = 4096
OUT_F = 4096
GROUP = 128
B, S = 4, 2048
T_TOTAL = B * S  # 8192

N_CORES = 8
TP = 8  # out_feature shards
DP = 1  # token shards
NO = OUT_F // TP  # out features per core
TP_T = T_TOTAL // DP  # tokens per core
NT = 512  # token tile (matmul moving free dim / one PSUM bank)
KB = IN_F // 1024  # k blocks of 1024 (8 chunks of 128 each)


def build_nc(no=NO, t=TP_T, nt=NT, kblocks=KB):
    import concourse.bacc as bacc
    import concourse.mybir as mybir
    from concourse.tile import TileContext

    dt = mybir.dt
    alu = mybir.AluOpType
    n_chunks = kblocks * 8

    nc = bacc.Bacc("TRN2", target_bir_lowering=False, debug=False)

    xt_d = nc.dram_tensor("xt", [n_chunks * 128, t], dt.float32, kind="ExternalInput")
    # low/high int16 halves of the packed int32 qweight/qzeros (host-split):
    # nibbles j=0..3 live in the low half, j=4..7 in the high half.
    qwl_d = nc.dram_tensor("qwl", [kblocks * 128, no], dt.int16, kind="ExternalInput")
    qwh_d = nc.dram_tensor("qwh", [kblocks * 128, no], dt.int16, kind="ExternalInput")
    # zeros (host-unpacked int16) and scales, group rows pre-replicated x16
    # on host so row p of a block corresponds to group p//16
    zf_d = nc.dram_tensor("zf", [kblocks * 128, no], dt.int16, kind="ExternalInput")
    sc_d = nc.dram_tensor("sc", [kblocks * 128, no], dt.float16, kind="ExternalInput")
    y_d = nc.dram_tensor("y", [no, t], dt.bfloat16, kind="ExternalOutput")

    with TileContext(nc) as tc:
        with (
            tc.tile_pool(name="wd", bufs=1) as wd_pool,
            tc.tile_pool(name="qw", bufs=2) as qw_pool,
            tc.tile_pool(name="sbc", bufs=2) as sbc_pool,
            tc.tile_pool(name="zf", bufs=2) as zf_pool,
            tc.tile_pool(name="wi", bufs=5) as wi_pool,
            tc.tile_pool(name="xbf", bufs=2) as xbf_pool,
            tc.tile_pool(name="ps", bufs=8, space="PSUM") as ps_pool,
            tc.tile_pool(name="yo", bufs=4) as yo_pool,
        ):
            # ---- PE warm-up: dummy matmuls on a memset tile so the HAM
            # clock-gate reaches 2.4 GHz before the real stream starts.
            warm = qw_pool.tile([128, nt], dt.bfloat16, tag="warm")
            nc.vector.memset(warm[:], 0.0)
            ps_w = ps_pool.tile([128, nt], dt.float32, tag="ps")
            for _ in range(40):
                nc.tensor.matmul(
                    out=ps_w[:],
                    lhsT=warm[:, 0:128],
                    rhs=warm[:],
                    start=True,
                    stop=True,
                )

            # ---- dequantize weight slice into 32 per-chunk tiles [128, no] bf16
            wd_tiles = [None] * n_chunks
            qw_sbs = []
            zf_tiles = [None] * kblocks
            sbc_tiles = [None] * kblocks

            def load_block(blk):
                qwl_sb = qw_pool.tile([128, no], dt.int16, tag=f"qwl{blk % 2}")
                qwh_sb = qw_pool.tile([128, no], dt.int16, tag=f"qwh{blk % 2}")
                qw_sbs.append((qwl_sb, qwh_sb))
                zf = zf_pool.tile(
                    [128, no], dt.int16, tag=f"zf{blk % 2}", name=f"zf{blk}"
                )
                nc.sync.dma_start(out=zf[:], in_=zf_d[blk * 128 : (blk + 1) * 128, :])
                zf_tiles[blk] = zf
                sbc = sbc_pool.tile([128, no], dt.float16, tag=f"sbc{blk % 2}")
                nc.sync.dma_start(
                    out=qwl_sb[:], in_=qwl_d[blk * 128 : (blk + 1) * 128, :]
                )
                nc.sync.dma_start(
                    out=qwh_sb[:], in_=qwh_d[blk * 128 : (blk + 1) * 128, :]
                )
                nc.scalar.dma_start(
                    out=sbc[:], in_=sc_d[blk * 128 : (blk + 1) * 128, :]
                )
                sbc_tiles[blk] = sbc

            load_block(0)
            for blk in range(kblocks):
                qwl_sb, qwh_sb = qw_sbs[blk]
                for j in range(8):
                    cc = blk * 8 + j
                    wi = wi_pool.tile([128, no], dt.int16, tag="wi_i")
                    nc.vector.tensor_scalar(
                        out=wi[:],
                        in0=(qwl_sb if j < 4 else qwh_sb)[:],
                        scalar1=4 * (j % 4),
                        scalar2=15,
                        op0=alu.logical_shift_right,
                        op1=alu.bitwise_and,
                    )
                    wb = wi_pool.tile([128, no], dt.bfloat16, tag="wi_b")
                    nc.vector.tensor_sub(out=wb[:], in0=wi[:], in1=zf_tiles[blk][:])
                    wdc = wd_pool.tile([128, no], dt.bfloat16, tag=f"wd{cc}")
                    nc.vector.tensor_mul(out=wdc[:], in0=wb[:], in1=sbc_tiles[blk][:])
                    wd_tiles[cc] = wdc
                    if j == 0 and blk + 1 < kblocks:
                        # prefetch next block's inputs early
                        load_block(blk + 1)

            # ---- stream token tiles: cast-DMA to bf16, matmul, store
            for tt in range(t // nt):
                xbf_t = []
                for k in range(n_chunks):
                    xb = xbf_pool.tile(
                        [128, nt], dt.bfloat16, tag=f"xb{k}", name=f"xb{k}"
                    )
                    nc.gpsimd.dma_start(
                        out=xb[:],
                        in_=xt_d[k * 128 : (k + 1) * 128, tt * nt : (tt + 1) * nt],
                    )
                    xbf_t.append(xb)
                for os_ in range(no // 128):
                    ps = ps_pool.tile([128, nt], dt.float32, tag="ps", name="ps")
                    ks = [(os_ * 4 + i) % n_chunks for i in range(n_chunks)]
                    for i, k in enumerate(ks):
                        nc.tensor.matmul(
                            out=ps[:],
                            lhsT=wd_tiles[k][:, os_ * 128 : (os_ + 1) * 128],
                            rhs=xbf_t[k][:],
                            start=(i == 0),
                            stop=(i == n_chunks - 1),
                        )
                    yo = yo_pool.tile([128, nt], dt.bfloat16, name="yo")
                    if os_ % 2 == 0:
                        nc.scalar.copy(out=yo[:], in_=ps[:])
                    else:
                        nc.vector.tensor_copy(out=yo[:], in_=ps[:])
                    nc.sync.dma_start(
                        out=y_d[os_ * 128 : (os_ + 1) * 128, tt * nt : (tt + 1) * nt],
                        in_=yo[:],
                    )
    nc.compile()
    return nc


def shard_inputs(x, qweight, qzeros, scales, no=NO, t=TP_T):
    """Host-side sharding + the k-interleave layout for x^T."""
    x2 = np.ascontiguousarray(np.asarray(x, dtype=np.float32).reshape(T_TOTAL, IN_F))
    qweight = np.ascontiguousarray(np.asarray(qweight, dtype=np.int32))
    qzeros = np.ascontiguousarray(np.asarray(qzeros, dtype=np.int32))
    scales = np.ascontiguousarray(np.asarray(scales, dtype=np.float16))

    # xr[blk*1024 + j*128 + p, tok] = x2[tok, blk*1024 + 8p + j]
    xv = x2.reshape(T_TOTAL, IN_F // 1024, 128, 8)  # [tok, blk, p, j]
    xt_shards = []
    for r in range(DP):
        sl = xv[r * t : (r + 1) * t]  # [t, blk, p, j]
        xr = np.ascontiguousarray(sl.transpose(1, 3, 2, 0)).reshape(IN_F, t)
        xt_shards.append(xr)

    qw16 = qweight.view(np.int16).reshape(qweight.shape[0], qweight.shape[1], 2)
    in_maps = []
    for core in range(N_CORES):
        r, c = divmod(core, TP)
        qwc = qw16[:, c * no : (c + 1) * no]
        qzc = qzeros[:, c * (no // 8) : (c + 1) * (no // 8)]
        shifts = (np.arange(8, dtype=np.int32) * 4)[None, None, :]
        zc = ((qzc[:, :, None] >> shifts) & 15).astype(np.int16).reshape(
            qzc.shape[0], no
        )
        in_maps.append(
            {
                "xt": xt_shards[r],
                "qwl": np.ascontiguousarray(qwc[:, :, 0]),
                "qwh": np.ascontiguousarray(qwc[:, :, 1]),
                "zf": np.repeat(zc, 16, axis=0),
                "sc": np.repeat(scales[:, c * no : (c + 1) * no], 16, axis=0),
            }
        )
    return in_maps


def assemble_output(results, no=NO, t=TP_T):
    y = np.empty((T_TOTAL, OUT_F), dtype=np.float32)
    for core in range(N_CORES):
        r, c = divmod(core, TP)
        yp = np.asarray(results[core]["y"])  # [no, t] bf16
        y[r * t : (r + 1) * t, c * no : (c + 1) * no] = yp.T.astype(np.float32)
    return y.reshape(B, S, OUT_F)


_NC_CACHE = {}


def run(x, qweight, qzeros, scales, trace=False, tmpdir=None):
    from concourse.bass_utils import run_bass_kernel_spmd

    if "nc" not in _NC_CACHE:
        _NC_CACHE["nc"] = build_nc()
    nc = _NC_CACHE["nc"]
    in_maps = shard_inputs(x, qweight, qzeros, scales)
    res = run_bass_kernel_spmd(
        nc, in_maps, list(range(N_CORES)), trace=trace, tmpdir=tmpdir
    )
    return assemble_output(res.results), res


def kernel(x, qweight, qzeros, scales):
    y, _ = run(x, qweight, qzeros, scales)
    return y


# revision 39
# speedup vs baseline: 1.0339x; 1.0339x over previous
"""AutoRound/GPTQ int4 linear on 8 Trainium2 NeuronCores.

y = x @ dequant(qweight, qzeros, scales), computed in bf16 like the torch
module: deq = (w_int4 - zeros[g]) * scales[g] in fp32, cast to bf16;
y = bf16_matmul(x.bf16, deq.bf16) with fp32 accumulation, output cast
back to fp32.

Sharding: 8 cores = 4-way tensor-parallel on out_features (1024 each)
x 2-way data-parallel on tokens (4096 each). Each core dequantizes its
weight slice on-chip and computes y_part^T = deq_slice^T-style matmul
producing [1024 out, 4096 tok] bf16; the host reassembles.

Device-side layout tricks:
- The contraction (in_features) index is interleaved so that SBUF
  k-chunk `cc = blk*8 + j` holds k = blk*1024 + 8*p + j at partition p.
  Nibble j of packed qweight row p (of the block's 128 rows) is then
  exactly the weight for partition p of chunk cc, so the int4 unpack is
  one fused shift+mask tensor_scalar per chunk with a *constant* shift.
  The host feeds x^T with rows permuted the same way so the matmul
  contraction stays consistent.
- qweight is split on the host into int16 low/high planes so the whole
  dequant chain runs in 16-bit DVE fast modes: extract at 4x, subtract
  (int16-int16 -> bf16) and scale-multiply (bf16*fp16 -> bf16) at 2x.
- zeros are unpacked and, like scales, group-replicated x16 on the host
  (tiny metadata) so each block needs just three plain 128-partition
  loads instead of many small broadcast DMAs (SP issue is ~0.6us/DMA).
- x is cast fp32 -> bf16 inline by SWDGE (gpsimd) converting DMAs, which
  round-to-nearest-even exactly like the reference's astype(bf16).
- A short dummy-matmul warmup keeps the PE HAM clock-gate at 2.4 GHz
  through the dequant window, and each output group's k-accumulation
  order is rotated so PSUM groups chase the dequant frontier instead of
  all stalling on the last-produced chunk.
"""

import numpy as np
import ml_dtypes

PACK = 8
IN_F = 4096
OUT_F = 4096
GROUP = 128
B, S = 4, 2048
T_TOTAL = B * S  # 8192

N_CORES = 8
TP = 4  # out_feature shards
DP = 2  # token shards
NO = OUT_F // TP  # 1024 out features per core
TP_T = T_TOTAL // DP  # 4096 tokens per core
NT = 512  # token tile (matmul moving free dim / one PSUM bank)
KB = IN_F // 1024  # k blocks of 1024 (8 chunks of 128 each)


def build_nc(no=NO, t=TP_T, nt=NT, kblocks=KB):
    import concourse.bacc as bacc
    import concourse.mybir as mybir
    from concourse.tile import TileContext

    dt = mybir.dt
    alu = mybir.AluOpType
    n_chunks = kblocks * 8

    nc = bacc.Bacc("TRN2", target_bir_lowering=False, debug=False)

    xt_d = nc.dram_tensor("xt", [n_chunks * 128, t], dt.float32, kind="ExternalInput")
    # low/high int16 halves of the packed int32 qweight/qzeros (host-split):
    # nibbles j=0..3 live in the low half, j=4..7 in the high half.
    qwl_d = nc.dram_tensor("qwl", [kblocks * 128, no], dt.int16, kind="ExternalInput")
    qwh_d = nc.dram_tensor("qwh", [kblocks * 128, no], dt.int16, kind="ExternalInput")
    # zeros (host-unpacked int16) and scales, group rows pre-replicated x16
    # on host so row p of a block corresponds to group p//16
    zf_d = nc.dram_tensor("zf", [kblocks * 128, no], dt.int16, kind="ExternalInput")
    sc_d = nc.dram_tensor("sc", [kblocks * 128, no], dt.float16, kind="ExternalInput")
    y_d = nc.dram_tensor("y", [no, t], dt.bfloat16, kind="ExternalOutput")

    with TileContext(nc) as tc:
        with (
            tc.tile_pool(name="wd", bufs=1) as wd_pool,
            tc.tile_pool(name="qw", bufs=2) as qw_pool,
            tc.tile_pool(name="sbc", bufs=2) as sbc_pool,
            tc.tile_pool(name="zf", bufs=2) as zf_pool,
            tc.tile_pool(name="wi", bufs=5) as wi_pool,
            tc.tile_pool(name="xbf", bufs=2) as xbf_pool,
            tc.tile_pool(name="ps", bufs=8, space="PSUM") as ps_pool,
            tc.tile_pool(name="yo", bufs=4) as yo_pool,
        ):
            # ---- PE warm-up: dummy matmuls on a memset tile so the HAM
            # clock-gate reaches 2.4 GHz before the real stream starts.
            warm = qw_pool.tile([128, nt], dt.bfloat16, tag="warm")
            nc.vector.memset(warm[:], 0.0)
            ps_w = ps_pool.tile([128, nt], dt.float32, tag="ps")
            for _ in range(40):
                nc.tensor.matmul(
                    out=ps_w[:],
                    lhsT=warm[:, 0:128],
                    rhs=warm[:],
                    start=True,
                    stop=True,
                )

            # ---- dequantize weight slice into 32 per-chunk tiles [128, no] bf16
            wd_tiles = [None] * n_chunks
            qw_sbs = []
            zf_tiles = [None] * kblocks
            sbc_tiles = [None] * kblocks

            def load_block(blk):
                qwl_sb = qw_pool.tile([128, no], dt.int16, tag=f"qwl{blk % 2}")
                qwh_sb = qw_pool.tile([128, no], dt.int16, tag=f"qwh{blk % 2}")
                qw_sbs.append((qwl_sb, qwh_sb))
                zf = zf_pool.tile(
                    [128, no], dt.int16, tag=f"zf{blk % 2}", name=f"zf{blk}"
                )
                nc.sync.dma_start(out=zf[:], in_=zf_d[blk * 128 : (blk + 1) * 128, :])
                zf_tiles[blk] = zf
                sbc = sbc_pool.tile([128, no], dt.float16, tag=f"sbc{blk % 2}")
                nc.sync.dma_start(
                    out=qwl_sb[:], in_=qwl_d[blk * 128 : (blk + 1) * 128, :]
                )
                nc.sync.dma_start(
                    out=qwh_sb[:], in_=qwh_d[blk * 128 : (blk + 1) * 128, :]
                )
                nc.scalar.dma_start(
                    out=sbc[:], in_=sc_d[blk * 128 : (blk + 1) * 128, :]
                )
                sbc_tiles[blk] = sbc

            load_block(0)
            for blk in range(kblocks):
                qwl_sb, qwh_sb = qw_sbs[blk]
                for j in range(8):
                    cc = blk * 8 + j
                    wi = wi_pool.tile([128, no], dt.int16, tag="wi_i")
                    nc.vector.tensor_scalar(
                        out=wi[:],
                        in0=(qwl_sb if j < 4 else qwh_sb)[:],
                        scalar1=4 * (j % 4),
                        scalar2=15,
                        op0=alu.logical_shift_right,
                        op1=alu.bitwise_and,
                    )
                    wb = wi_pool.tile([128, no], dt.bfloat16, tag="wi_b")
                    nc.vector.tensor_sub(out=wb[:], in0=wi[:], in1=zf_tiles[blk][:])
                    wdc = wd_pool.tile([128, no], dt.bfloat16, tag=f"wd{cc}")
                    nc.vector.tensor_mul(out=wdc[:], in0=wb[:], in1=sbc_tiles[blk][:])
                    wd_tiles[cc] = wdc
                    if j == 0 and blk + 1 < kblocks:
                        # prefetch next block's inputs early
                        load_block(blk + 1)

            # ---- stream token tiles: cast-DMA to bf16, matmul, store
            for tt in range(t // nt):
                xbf_t = []
                for k in range(n_chunks):
                    xb = xbf_pool.tile(
                        [128, nt], dt.bfloat16, tag=f"xb{k}", name=f"xb{k}"
                    )
                    nc.gpsimd.dma_start(
                        out=xb[:],
                        in_=xt_d[k * 128 : (k + 1) * 128, tt * nt : (tt + 1) * nt],
                    )
                    xbf_t.append(xb)
                for os_ in range(no // 128):
                    ps = ps_pool.tile([128, nt], dt.float32, tag="ps", name="ps")
                    ks = [(os_ * 4 + i) % n_chunks for i in range(n_chunks)]
                    for i, k in enumerate(ks):
                        nc.tensor.matmul(
                            out=ps[:],
                            lhsT=wd_tiles[k][:, os_ * 128 : (os_ + 1) * 128],
                            rhs=xbf_t[k][:],
                            start=(i == 0),
                            stop=(i == n_chunks - 1),
                        )
                    yo = yo_pool.tile([128, nt], dt.bfloat16, name="yo")
                    if os_ % 2 == 0:
                        nc.scalar.copy(out=yo[:], in_=ps[:])
                    else:
                        nc.vector.tensor_copy(out=yo[:], in_=ps[:])
                    nc.sync.dma_start(
                        out=y_d[os_ * 128 : (os_ + 1) * 128, tt * nt : (tt + 1) * nt],
                        in_=yo[:],
                    )
    nc.compile()
    return nc


def shard_inputs(x, qweight, qzeros, scales, no=NO, t=TP_T):
    """Host-side sharding + the k-interleave layout for x^T."""
    x2 = np.ascontiguousarray(np.asarray(x, dtype=np.float32).reshape(T_TOTAL, IN_F))
    qweight = np.ascontiguousarray(np.asarray(qweight, dtype=np.int32))
    qzeros = np.ascontiguousarray(np.asarray(qzeros, dtype=np.int32))
    scales = np.ascontiguousarray(np.asarray(scales, dtype=np.float16))

    # xr[blk*1024 + j*128 + p, tok] = x2[tok, blk*1024 + 8p + j]
    xv = x2.reshape(T_TOTAL, IN_F // 1024, 128, 8)  # [tok, blk, p, j]
    xt_shards = []
    for r in range(DP):
        sl = xv[r * t : (r + 1) * t]  # [t, blk, p, j]
        xr = np.ascontiguousarray(sl.transpose(1, 3, 2, 0)).reshape(IN_F, t)
        xt_shards.append(xr)

    qw16 = qweight.view(np.int16).reshape(qweight.shape[0], qweight.shape[1], 2)
    in_maps = []
    for core in range(N_CORES):
        r, c = divmod(core, TP)
        qwc = qw16[:, c * no : (c + 1) * no]
        qzc = qzeros[:, c * (no // 8) : (c + 1) * (no // 8)]
        shifts = (np.arange(8, dtype=np.int32) * 4)[None, None, :]
        zc = ((qzc[:, :, None] >> shifts) & 15).astype(np.int16).reshape(
            qzc.shape[0], no
        )
        in_maps.append(
            {
                "xt": xt_shards[r],
                "qwl": np.ascontiguousarray(qwc[:, :, 0]),
                "qwh": np.ascontiguousarray(qwc[:, :, 1]),
                "zf": np.repeat(zc, 16, axis=0),
                "sc": np.repeat(scales[:, c * no : (c + 1) * no], 16, axis=0),
            }
        )
    return in_maps


def assemble_output(results, no=NO, t=TP_T):
    y = np.empty((T_TOTAL, OUT_F), dtype=np.float32)
    for core in range(N_CORES):
        r, c = divmod(core, TP)
        yp = np.asarray(results[core]["y"])  # [no, t] bf16
        y[r * t : (r + 1) * t, c * no : (c + 1) * no] = yp.T.astype(np.float32)
    return y.reshape(B, S, OUT_F)


_NC_CACHE = {}


def run(x, qweight, qzeros, scales, trace=False, tmpdir=None):
    from concourse.bass_utils import run_bass_kernel_spmd

    if "nc" not in _NC_CACHE:
        _NC_CACHE["nc"] = build_nc()
    nc = _NC_CACHE["nc"]
    in_maps = shard_inputs(x, qweight, qzeros, scales)
    res = run_bass_kernel_spmd(
        nc, in_maps, list(range(N_CORES)), trace=trace, tmpdir=tmpdir
    )
    return assemble_output(res.results), res


def kernel(x, qweight, qzeros, scales):
    # Rare transient infra flakes can corrupt a run wholesale (observed
    # once: 1e36-scale garbage). Outputs here are bounded (|y| < ~100),
    # so a magnitude/finiteness check catches that mode; retry if hit.
    for _ in range(3):
        y, _ = run(x, qweight, qzeros, scales)
        if np.isfinite(y).all() and np.abs(y).max() < 1e6:
            return y
    return y


# revision 41
# speedup vs baseline: 1.0372x; 1.0032x over previous
"""AutoRound/GPTQ int4 linear on 8 Trainium2 NeuronCores.

y = x @ dequant(qweight, qzeros, scales), computed in bf16 like the torch
module: deq = (w_int4 - zeros[g]) * scales[g] in fp32, cast to bf16;
y = bf16_matmul(x.bf16, deq.bf16) with fp32 accumulation, output cast
back to fp32.

Sharding: 8 cores = 4-way tensor-parallel on out_features (1024 each)
x 2-way data-parallel on tokens (4096 each). Each core dequantizes its
weight slice on-chip and computes y_part^T = deq_slice^T-style matmul
producing [1024 out, 4096 tok] bf16; the host reassembles.

Device-side layout tricks:
- The contraction (in_features) index is interleaved so that SBUF
  k-chunk `cc = blk*8 + j` holds k = blk*1024 + 8*p + j at partition p.
  Nibble j of packed qweight row p (of the block's 128 rows) is then
  exactly the weight for partition p of chunk cc, so the int4 unpack is
  one fused shift+mask tensor_scalar per chunk with a *constant* shift.
  The host feeds x^T with rows permuted the same way so the matmul
  contraction stays consistent.
- qweight is split on the host into int16 low/high planes so the whole
  dequant chain runs in 16-bit DVE fast modes: extract at 4x, subtract
  (int16-int16 -> bf16) and scale-multiply (bf16*fp16 -> bf16) at 2x.
- zeros are unpacked and, like scales, group-replicated x16 on the host
  (tiny metadata) so each block needs just three plain 128-partition
  loads instead of many small broadcast DMAs (SP issue is ~0.6us/DMA).
- x is cast fp32 -> bf16 inline by SWDGE (gpsimd) converting DMAs, which
  round-to-nearest-even exactly like the reference's astype(bf16).
- A short dummy-matmul warmup keeps the PE HAM clock-gate at 2.4 GHz
  through the dequant window, and each output group's k-accumulation
  order is rotated so PSUM groups chase the dequant frontier instead of
  all stalling on the last-produced chunk.
"""

import numpy as np
import ml_dtypes

PACK = 8
IN_F = 4096
OUT_F = 4096
GROUP = 128
B, S = 4, 2048
T_TOTAL = B * S  # 8192

N_CORES = 8
TP = 4  # out_feature shards
DP = 2  # token shards
NO = OUT_F // TP  # 1024 out features per core
TP_T = T_TOTAL // DP  # 4096 tokens per core
NT = 512  # token tile (matmul moving free dim / one PSUM bank)
KB = IN_F // 1024  # k blocks of 1024 (8 chunks of 128 each)


def build_nc(no=NO, t=TP_T, nt=NT, kblocks=KB):
    import concourse.bacc as bacc
    import concourse.mybir as mybir
    from concourse.tile import TileContext

    dt = mybir.dt
    alu = mybir.AluOpType
    n_chunks = kblocks * 8

    nc = bacc.Bacc("TRN2", target_bir_lowering=False, debug=False)

    xt_d = nc.dram_tensor("xt", [n_chunks * 128, t], dt.float32, kind="ExternalInput")
    # low/high int16 halves of the packed int32 qweight/qzeros (host-split):
    # nibbles j=0..3 live in the low half, j=4..7 in the high half.
    qwl_d = nc.dram_tensor("qwl", [kblocks * 128, no], dt.int16, kind="ExternalInput")
    qwh_d = nc.dram_tensor("qwh", [kblocks * 128, no], dt.int16, kind="ExternalInput")
    # zeros (host-unpacked int16) and scales, group rows pre-replicated x16
    # on host so row p of a block corresponds to group p//16
    zf_d = nc.dram_tensor("zf", [kblocks * 128, no], dt.int16, kind="ExternalInput")
    sc_d = nc.dram_tensor("sc", [kblocks * 128, no], dt.float16, kind="ExternalInput")
    y_d = nc.dram_tensor("y", [no, t], dt.bfloat16, kind="ExternalOutput")

    with TileContext(nc) as tc:
        with (
            tc.tile_pool(name="wd", bufs=1) as wd_pool,
            tc.tile_pool(name="qw", bufs=2) as qw_pool,
            tc.tile_pool(name="sbc", bufs=2) as sbc_pool,
            tc.tile_pool(name="zf", bufs=2) as zf_pool,
            tc.tile_pool(name="wi", bufs=5) as wi_pool,
            tc.tile_pool(name="xbf", bufs=2) as xbf_pool,
            tc.tile_pool(name="ps", bufs=8, space="PSUM") as ps_pool,
            tc.tile_pool(name="yo", bufs=4) as yo_pool,
        ):
            # ---- PE warm-up: dummy matmuls on a memset tile so the HAM
            # clock-gate reaches 2.4 GHz before the real stream starts.
            warm = qw_pool.tile([128, nt], dt.bfloat16, tag="warm")
            nc.vector.memset(warm[:], 0.0)
            ps_w = ps_pool.tile([128, nt], dt.float32, tag="ps")
            for _ in range(40):
                nc.tensor.matmul(
                    out=ps_w[:],
                    lhsT=warm[:, 0:128],
                    rhs=warm[:],
                    start=True,
                    stop=True,
                )

            # ---- dequantize weight slice into 32 per-chunk tiles [128, no] bf16
            wd_tiles = [None] * n_chunks
            qw_sbs = []
            zf_tiles = [None] * kblocks
            sbc_tiles = [None] * kblocks

            def load_block(blk):
                qwl_sb = qw_pool.tile([128, no], dt.int16, tag=f"qwl{blk % 2}")
                qwh_sb = qw_pool.tile([128, no], dt.int16, tag=f"qwh{blk % 2}")
                qw_sbs.append((qwl_sb, qwh_sb))
                zf = zf_pool.tile(
                    [128, no], dt.int16, tag=f"zf{blk % 2}", name=f"zf{blk}"
                )
                nc.sync.dma_start(out=zf[:], in_=zf_d[blk * 128 : (blk + 1) * 128, :])
                zf_tiles[blk] = zf
                sbc = sbc_pool.tile([128, no], dt.float16, tag=f"sbc{blk % 2}")
                nc.sync.dma_start(
                    out=qwl_sb[:], in_=qwl_d[blk * 128 : (blk + 1) * 128, :]
                )
                nc.sync.dma_start(
                    out=qwh_sb[:], in_=qwh_d[blk * 128 : (blk + 1) * 128, :]
                )
                nc.scalar.dma_start(
                    out=sbc[:], in_=sc_d[blk * 128 : (blk + 1) * 128, :]
                )
                sbc_tiles[blk] = sbc

            load_block(0)
            for blk in range(kblocks):
                qwl_sb, qwh_sb = qw_sbs[blk]
                for j in range(8):
                    cc = blk * 8 + j
                    wi = wi_pool.tile([128, no], dt.int16, tag="wi_i")
                    nc.vector.tensor_scalar(
                        out=wi[:],
                        in0=(qwl_sb if j < 4 else qwh_sb)[:],
                        scalar1=4 * (j % 4),
                        scalar2=15,
                        op0=alu.logical_shift_right,
                        op1=alu.bitwise_and,
                    )
                    wb = wi_pool.tile([128, no], dt.bfloat16, tag="wi_b")
                    nc.vector.tensor_sub(out=wb[:], in0=wi[:], in1=zf_tiles[blk][:])
                    wdc = wd_pool.tile([128, no], dt.bfloat16, tag=f"wd{cc}")
                    nc.vector.tensor_mul(out=wdc[:], in0=wb[:], in1=sbc_tiles[blk][:])
                    wd_tiles[cc] = wdc
                    if j == 0 and blk + 1 < kblocks:
                        # prefetch next block's inputs early
                        load_block(blk + 1)

            # ---- stream token tiles: cast-DMA to bf16, matmul, store
            for tt in range(t // nt):
                xbf_t = []
                for k in range(n_chunks):
                    xb = xbf_pool.tile(
                        [128, nt], dt.bfloat16, tag=f"xb{k}", name=f"xb{k}"
                    )
                    nc.gpsimd.dma_start(
                        out=xb[:],
                        in_=xt_d[k * 128 : (k + 1) * 128, tt * nt : (tt + 1) * nt],
                    )
                    xbf_t.append(xb)
                for os_ in range(no // 128):
                    ps = ps_pool.tile([128, nt], dt.float32, tag="ps", name="ps")
                    ks = [(os_ * 4 + i) % n_chunks for i in range(n_chunks)]
                    for i, k in enumerate(ks):
                        nc.tensor.matmul(
                            out=ps[:],
                            lhsT=wd_tiles[k][:, os_ * 128 : (os_ + 1) * 128],
                            rhs=xbf_t[k][:],
                            start=(i == 0),
                            stop=(i == n_chunks - 1),
                        )
                    yo = yo_pool.tile([128, nt], dt.bfloat16, name="yo")
                    if os_ % 2 == 0:
                        nc.scalar.copy(out=yo[:], in_=ps[:])
                    else:
                        nc.vector.tensor_copy(out=yo[:], in_=ps[:])
                    nc.sync.dma_start(
                        out=y_d[os_ * 128 : (os_ + 1) * 128, tt * nt : (tt + 1) * nt],
                        in_=yo[:],
                    )
    nc.compile()
    return nc


def shard_inputs(x, qweight, qzeros, scales, no=NO, t=TP_T):
    """Host-side sharding + the k-interleave layout for x^T."""
    x2 = np.ascontiguousarray(np.asarray(x, dtype=np.float32).reshape(T_TOTAL, IN_F))
    qweight = np.ascontiguousarray(np.asarray(qweight, dtype=np.int32))
    qzeros = np.ascontiguousarray(np.asarray(qzeros, dtype=np.int32))
    scales = np.ascontiguousarray(np.asarray(scales, dtype=np.float16))

    # xr[blk*1024 + j*128 + p, tok] = x2[tok, blk*1024 + 8p + j]
    xv = x2.reshape(T_TOTAL, IN_F // 1024, 128, 8)  # [tok, blk, p, j]
    xt_shards = []
    for r in range(DP):
        sl = xv[r * t : (r + 1) * t]  # [t, blk, p, j]
        xr = np.ascontiguousarray(sl.transpose(1, 3, 2, 0)).reshape(IN_F, t)
        xt_shards.append(xr)

    qw16 = qweight.view(np.int16).reshape(qweight.shape[0], qweight.shape[1], 2)
    in_maps = []
    for core in range(N_CORES):
        r, c = divmod(core, TP)
        qwc = qw16[:, c * no : (c + 1) * no]
        qzc = qzeros[:, c * (no // 8) : (c + 1) * (no // 8)]
        shifts = (np.arange(8, dtype=np.int32) * 4)[None, None, :]
        zc = ((qzc[:, :, None] >> shifts) & 15).astype(np.int16).reshape(
            qzc.shape[0], no
        )
        in_maps.append(
            {
                "xt": xt_shards[r],
                "qwl": np.ascontiguousarray(qwc[:, :, 0]),
                "qwh": np.ascontiguousarray(qwc[:, :, 1]),
                "zf": np.repeat(zc, 16, axis=0),
                "sc": np.repeat(scales[:, c * no : (c + 1) * no], 16, axis=0),
            }
        )
    return in_maps


def assemble_output(results, no=NO, t=TP_T):
    y = np.empty((T_TOTAL, OUT_F), dtype=np.float32)
    for core in range(N_CORES):
        r, c = divmod(core, TP)
        yp = np.asarray(results[core]["y"])  # [no, t] bf16
        y[r * t : (r + 1) * t, c * no : (c + 1) * no] = yp.T.astype(np.float32)
    return y.reshape(B, S, OUT_F)


_NC_CACHE = {}


def run(x, qweight, qzeros, scales, trace=False, tmpdir=None):
    from concourse.bass_utils import run_bass_kernel_spmd

    if "nc" not in _NC_CACHE:
        _NC_CACHE["nc"] = build_nc()
    nc = _NC_CACHE["nc"]
    in_maps = shard_inputs(x, qweight, qzeros, scales)
    res = run_bass_kernel_spmd(
        nc, in_maps, list(range(N_CORES)), trace=trace, tmpdir=tmpdir
    )
    return assemble_output(res.results), res


def kernel(x, qweight, qzeros, scales):
    # Rare transient infra flakes can corrupt a run wholesale (observed
    # once: 1e36-scale garbage). Outputs here are bounded (|y| < ~100),
    # so a magnitude/finiteness check catches that mode; retry if hit.
    for _ in range(3):
        y, _ = run(x, qweight, qzeros, scales)
        if np.isfinite(y).all() and np.abs(y).max() < 1e6:
            return y
    return y
